# revision 4
# baseline (speedup 1.0000x reference)
"""Trainium2 Bass kernel for nn_Jitter: out[:, i, :] = x[:, indices[i], :].

Full shapes: x (64, 4096, 256) f32, indices (4096,) int64 -> out (64, 4096, 256) f32.

Strategy: data-parallel over batch dim across 8 NeuronCores (8 batches per
core); the tiny index vector is replicated to every core. On each core the
time-axis gather uses the SWDGE `dma_gather` ucode instruction: per batch,
4 x 1024-descriptor gathers (the SWDGE descriptor-ring limit per
instruction) pull all 4096 rows (1KB each) of one batch into a
[128, 32, 256] SBUF tile; each gather's slice is stored contiguously to
out[b] by its own HWDGE DMA as soon as it lands. Memory-bound: each core
moves 32MB in + 32MB out.

Key performance choices (HW-measured on trn2):
* 4 SWDGE queues with gathers round-robined across them: 302us -> 216us
  per 64MB iteration (SWDGE single-queue serialization was the bottleneck).
* Host-side index permutation (idx'[j*128+p] = idx[p*32+j]) makes gathered
  row t=p*32+j land at SBUF slot [p, j], so stores are contiguous writes
  and gather reads are near-sequential per SDMA engine for jitter-like
  index vectors.
* Quarter-stores alternating the two HWDGE rings (SP/Activation): each
  1MB store depends only on its own gather, ~4us over one 4MB store.
* Indices are clipped to [0, T-1], matching jnp.take's default clip mode.
"""

import numpy as np

import concourse.bass as bass
import concourse.tile as tile
from concourse import bacc, mybir
from concourse.bass_utils import run_bass_kernel_spmd
from concourse.library_config import mlp as _mlp_lib

N_CORES = 8
B, T, C = 64, 4096, 256
B_LOC = B // N_CORES  # 8 batches per core
P = 128               # SBUF partitions
J = T // P            # 32 gathered rows per partition
JW = T // 16          # idx tile cols (16-partition wrap)

_CACHE = {}

GSPLIT = 4            # 1024 descriptors per dma_gather (SWDGE ring limit)
NQ = 4                # SWDGE queues; gathers round-robin across them
IDX_PER_G = T // GSPLIT
JW_PER_G = JW // GSPLIT
J_PER_G = J // GSPLIT


def _build(repeat: int = 1, bufs: int = 4):
    """Build + compile the per-core SPMD program."""
    nc = bacc.Bacc("TRN2", target_bir_lowering=False, debug=False,
                   num_devices=N_CORES, num_swdge_queues=NQ)
    x_ext = nc.dram_tensor("x", [B_LOC, T, C], mybir.dt.float32,
                           kind="ExternalInput").ap()
    idx_ext = nc.dram_tensor("idx", [P, JW], mybir.dt.int16,
                             kind="ExternalInput").ap()
    out_ext = nc.dram_tensor("out", [B_LOC, T, C], mybir.dt.float32,
                             kind="ExternalOutput").ap()

    with tile.TileContext(nc) as tc:
        with tc.tile_pool(name="idxp", bufs=1) as idx_pool, \
             tc.tile_pool(name="data", bufs=bufs) as data_pool:
            nc.gpsimd.load_library(_mlp_lib)
            idx_t = idx_pool.tile([P, JW], mybir.dt.int16)
            nc.sync.dma_start(out=idx_t[:], in_=idx_ext[:])
            qn = 0
            for _ in range(repeat):
                for b in range(B_LOC):
                    dt = data_pool.tile([P, J, C], mybir.dt.float32)
                    for g in range(GSPLIT):
                        # permuted index n = j*128 + p gathers out-row p*32+j
                        # into slot [p, j]; gather g covers j in
                        # [g*J_PER_G, (g+1)*J_PER_G).
                        nc.gpsimd.dma_gather(
                            dt[:, g * J_PER_G:(g + 1) * J_PER_G, :],
                            x_ext[b],
                            idx_t[:, g * JW_PER_G:(g + 1) * JW_PER_G],
                            num_idxs=IDX_PER_G, num_idxs_reg=IDX_PER_G,
                            elem_size=C,
                            queue_num=qn % NQ,
                        )
                        qn += 1
                    # slot [p, j] holds out row p*32+j: contiguous stores,
                    # one per gather slice so each fires as its gather lands,
                    # alternating the two HWDGE rings (SP / Activation)
                    ov = out_ext[b].rearrange("(p j) c -> p j c", p=P)
                    for g in range(GSPLIT):
                        eng = nc.scalar if g % 2 else nc.sync
                        eng.dma_start(
                            out=ov[:, g * J_PER_G:(g + 1) * J_PER_G, :],
                            in_=dt[:, g * J_PER_G:(g + 1) * J_PER_G, :])
    nc.compile()
    return nc


def _prep_idx(indices: np.ndarray) -> np.ndarray:
    idx16 = np.clip(indices, 0, T - 1).astype(np.int16)
    perm = np.ascontiguousarray(idx16.reshape(P, J).T).reshape(-1)
    wrapped = np.ascontiguousarray(perm.reshape(JW, 16).T)       # [16, JW]
    return np.ascontiguousarray(np.tile(wrapped, (P // 16, 1)))  # [128, JW]


def kernel(x: np.ndarray, indices: np.ndarray) -> np.ndarray:
    if "main" not in _CACHE:
        _CACHE["main"] = _build()
    nc = _CACHE["main"]

    idx_arr = _prep_idx(np.asarray(indices))
    x = np.asarray(x)
    in_maps = [
        {"x": np.ascontiguousarray(x[i * B_LOC:(i + 1) * B_LOC]),
         "idx": idx_arr}
        for i in range(N_CORES)
    ]
    res = run_bass_kernel_spmd(nc, in_maps, list(range(N_CORES)))
    return np.concatenate([res.results[i]["out"] for i in range(N_CORES)],
                          axis=0)
